# revision 1
# baseline (speedup 1.0000x reference)
"""Trainium2 Bass kernel for nn_GCN_26242250179008.

The reference model is a ChebConv(K=1) stack, which degenerates to plain
dense linear layers (edge_index is never used):

    h = relu(x @ W1.T + b1); h = relu(h @ W2.T + b2); h = h @ W3.T + b3
    g = mean(h, axis=0); out = log_softmax(g @ Wl.T + bl)

Because layer 3 is linear (no relu), mean() commutes with it:
    mean(h3) = mean(h2) @ W3.T + b3
so the device only needs sum_n relu(W2 @ relu(W1 @ x_n + b1) + b2) — a
[128] partial per core.  Layer 3, the classifier head and log_softmax are
O(128^2) and run on host.

Sharding: data-parallel over nodes.  x is split row-wise into 8 shards of
25000 nodes; each shard is transposed on host to [128 features, 25000
nodes] so features sit on SBUF partitions and the matmuls need no
on-device transpose.  The small weights are replicated.  Per-core output
is a [128, 1] partial sum; host reduces across cores (cheaper than an
all-reduce for 512 bytes).
"""

import math
import os

import numpy as np

N_NODES = 200_000
F = 128
N_CORES = 8
PER_CORE = N_NODES // N_CORES  # 25000
CHUNK = 4096  # nodes per DMA chunk
GROUP = 1024  # nodes per relu/accum group (2 PSUM banks)
MM_N = 512    # matmul moving-operand free dim (1 PSUM bank, fp32)

# variant config: "bf16" (fast, rel err ~2.5e-05) or "f32r" (fp32-precision,
# rel err ~2e-06, ~20% slower).  bf16 halves the HBM traffic for x (the
# memory-bound term) and runs the PE at full rate; the mean-pool over 200k
# nodes averages the quantization noise away.
VARIANT = os.environ.get("GCN_VARIANT", "bf16")

_COMPILED = {}


def _build_program(variant):
    from concourse import bacc, mybir, tile

    f32 = mybir.dt.float32
    in_dt = {"f32r": mybir.dt.float32r, "bf16": mybir.dt.bfloat16}[variant]

    nc = bacc.Bacc(None, target_bir_lowering=False, debug=False)

    xt = nc.dram_tensor("xt", [F, PER_CORE], in_dt, kind="ExternalInput")
    wpk = nc.dram_tensor("wpack", [F, 2 * F], in_dt, kind="ExternalInput")
    bpk = nc.dram_tensor("bpack", [F, 2], f32, kind="ExternalInput")
    out = nc.dram_tensor("partial", [F, 1], f32, kind="ExternalOutput")

    n_groups = math.ceil(PER_CORE / GROUP)

    with tile.TileContext(nc, pool_alloc_mode="queue") as tc:
        with (
            tc.tile_pool(name="const", bufs=1) as cpool,
            tc.tile_pool(name="h1", bufs=6) as h1pool,
            tc.tile_pool(name="h2", bufs=4) as h2pool,
            tc.tile_pool(name="ps1", bufs=2, space="PSUM") as ps1pool,
            tc.tile_pool(name="ps2", bufs=2, space="PSUM") as ps2pool,
        ):
            wpk_sb = cpool.tile([F, 2 * F], in_dt)
            bpk_sb = cpool.tile([F, 2], f32)
            zero_sb = cpool.tile([F, GROUP], f32)
            nc.vector.memset(zero_sb[:], 0.0)

            acc = cpool.tile([F, n_groups], f32)

            chunk_starts = []
            pos = 0
            for w in (1024, 1024, 2048):  # ramp-up chunks
                chunk_starts.append((pos, w))
                pos += w
            while pos < PER_CORE:
                w = min(CHUNK, PER_CORE - pos)
                chunk_starts.append((pos, w))
                pos += w

            # One persistent x buffer for the whole shard: chunked DMAs land
            # in disjoint slices, so no pool rotation / slot sems for x.
            x_all = cpool.tile([F, PER_CORE], in_dt)
            # chunk 0 goes down the Scalar HWDGE ring so it transfers in
            # parallel with the Sync ring's chunk 1 + weights.
            (s0, w0), (s1, w1) = chunk_starts[:2]
            nc.scalar.dma_start(x_all[:, s0 : s0 + w0], xt[:, s0 : s0 + w0])
            nc.sync.dma_start(x_all[:, s1 : s1 + w1], xt[:, s1 : s1 + w1])
            nc.sync.dma_start(wpk_sb[:], wpk[:])
            nc.sync.dma_start(bpk_sb[:], bpk[:])
            w1_sb = wpk_sb[:, 0:F]
            w2_sb = wpk_sb[:, F : 2 * F]
            b1_sb = bpk_sb[:, 0:1]
            b2_sb = bpk_sb[:, 1:2]

            gidx = 0
            for ci, (start, width) in enumerate(chunk_starts):
                if ci >= 2:
                    nc.sync.dma_start(
                        x_all[:, start : start + width],
                        xt[:, start : start + width],
                    )
                xt_sb = x_all[:, start : start + width]
                for g0 in range(0, width, GROUP):
                    gw = min(GROUP, width - g0)
                    ps1 = ps1pool.tile([F, GROUP], f32, tag="ps1")
                    for j in range(0, gw, MM_N):
                        jw = min(MM_N, gw - j)
                        nc.tensor.matmul(
                            ps1[:, j : j + jw],
                            w1_sb,
                            xt_sb[:, g0 + j : g0 + j + jw],
                            start=True,
                            stop=True,
                        )
                    h1 = h1pool.tile([F, GROUP], in_dt, tag="h1")
                    # h1 = relu(ps1 + b1) on ScalarE
                    nc.scalar.activation(
                        h1[:, :gw],
                        ps1[:, :gw],
                        mybir.ActivationFunctionType.Relu,
                        bias=b1_sb,
                    )
                    ps2 = ps2pool.tile([F, GROUP], f32, tag="ps2")
                    for j in range(0, gw, MM_N):
                        jw = min(MM_N, gw - j)
                        nc.tensor.matmul(
                            ps2[:, j : j + jw],
                            w2_sb,
                            h1[:, j : j + jw],
                            start=True,
                            stop=True,
                        )
                    h2 = h2pool.tile([F, GROUP], f32, tag="h2")
                    # relu2 + row-sum.  DVE carries most groups (inline
                    # accum); three groups go to ScalarE (activation accum)
                    # to balance the engines (~29.5us each).
                    if gidx in (8, 20):
                        nc.scalar.activation(
                            h2[:, :gw],
                            ps2[:, :gw],
                            mybir.ActivationFunctionType.Relu,
                            bias=b2_sb,
                            accum_out=acc[:, gidx : gidx + 1],
                        )
                    else:
                        nc.vector.scalar_tensor_tensor(
                            h2[:, :gw],
                            ps2[:, :gw],
                            b2_sb,
                            zero_sb[:, :gw],
                            op0=mybir.AluOpType.add,
                            op1=mybir.AluOpType.max,
                            accum_out=acc[:, gidx : gidx + 1],
                        )
                    gidx += 1
            assert gidx == n_groups

            partial_sb = cpool.tile([F, 1], f32)
            nc.vector.tensor_reduce(
                partial_sb[:],
                acc[:],
                axis=mybir.AxisListType.X,
                op=mybir.AluOpType.add,
            )
            nc.sync.dma_start(out[:], partial_sb[:])

    nc.compile()
    return nc


def _get_program(variant=None):
    variant = variant or VARIANT
    if variant not in _COMPILED:
        _COMPILED[variant] = _build_program(variant)
    return _COMPILED[variant]


def _run_on_device(in_maps, variant=None, **kwargs):
    from concourse.bass_utils import run_bass_kernel_spmd

    nc = _get_program(variant)
    return run_bass_kernel_spmd(nc, in_maps, core_ids=list(range(N_CORES)), **kwargs)


def _in_dtype(variant=None):
    variant = variant or VARIANT
    if variant == "bf16":
        import ml_dtypes

        return np.dtype(ml_dtypes.bfloat16)
    return np.dtype(np.float32)


def _make_in_maps(x, W1, b1, W2, b2, variant=None):
    dt = _in_dtype(variant)
    x = np.ascontiguousarray(np.asarray(x, dtype=np.float32)).reshape(N_NODES, F)
    shards = x.reshape(N_CORES, PER_CORE, F)
    wpack = np.concatenate(
        [np.asarray(W1, np.float32).T, np.asarray(W2, np.float32).T], axis=1
    ).astype(dt)
    bpack = np.stack(
        [np.asarray(b1, np.float32), np.asarray(b2, np.float32)], axis=1
    )
    in_maps = []
    for c in range(N_CORES):
        in_maps.append(
            {
                "xt": np.ascontiguousarray(shards[c].T).astype(dt),
                "wpack": wpack,
                "bpack": bpack,
            }
        )
    return in_maps


def _host_head(partials, W3, b3, Wl, bl):
    # partials: [N_CORES, 128] fp32 sums of h2 over each shard.
    g = partials.astype(np.float64).sum(axis=0) / float(N_NODES)  # mean(h2)
    z = np.asarray(W3, np.float64) @ g + np.asarray(b3, np.float64)
    logits = np.asarray(Wl, np.float64) @ z + np.asarray(bl, np.float64)
    m = logits.max()
    ls = logits - (m + np.log(np.exp(logits - m).sum()))
    return ls[None, :].astype(np.float32)


def kernel(x, edge_index, W1, b1, W2, b2, W3, b3, Wl, bl, **_unused):
    # edge_index is unused by the reference computation (ChebConv K=1).
    in_maps = _make_in_maps(x, W1, b1, W2, b2)
    res = _run_on_device(in_maps)
    partials = np.stack(
        [np.asarray(r["partial"], np.float32).reshape(F) for r in res.results]
    )
    return _host_head(partials, W3, b3, Wl, bl)

